# revision 4
# baseline (speedup 1.0000x reference)
"""CAM (channel attention) module kernel for Trainium2, 8-core data-parallel.

Reference computation (per sample, C=512, HW=4096):
    v = x.reshape(C, HW)
    E = v @ v.T                                  # (C, C)
    att = softmax(rowmax(E) - E, axis=-1)        # == softmax(-E) stabilized at rowmin
    o = att @ v                                  # (C, HW)
    o = softmax(o, axis=-1)
    out = x + gamma * o
Sharding: data-parallel over batch B=16 -> 2 samples per NeuronCore.

v2 changes vs the 160 us baseline (target: the ~95 us HBM roofline):
- vb (natural-layout fp8 v) is produced by GpSimd tensor_copy casts instead of
  SWDGE cast-DMAs: removes ~21 MB of SBUF->SBUF traffic from the DMA queues,
  which were the co-bottleneck (~41% of every queue's busy time).
- Sample 1's v^T transposes consume the fp8 vb tiles (FWL weight loads + fp8
  moving) instead of fp32 x: less than half the PE time.  Sample 0 keeps fp32
  transposes since they overlap the initial load phase where PE is idle.
- Transpose pairs land in one [P,1024] PSUM tile and are evicted with a single
  copy (halves the per-instruction "read-write bubble" overhead).
- One shared PSUM pool ([P,1024] f32-sized, bufs=4 = all 8 banks).
- xf pool has 16 bufs so sample 1's loads never wait on sample 0's stores.
- Final residual adds split DVE/GpSimd so stores start flowing early.
"""

import sys

if "/opt/trn_rl_repo" not in sys.path:
    sys.path.insert(0, "/opt/trn_rl_repo")

from contextlib import ExitStack

import numpy as np

P = 128
C = 512
HW = 4096
HHW = HW // 2  # 2048: half-width x tiles
S = 2  # samples per core
CB = C // P  # 4 channel blocks
NB = HW // P  # 32 spatial blocks
NT = NB // 2  # 16 DoubleRow k-pairs for matmul 1
NJ = HW // 1024  # 4 psum chunks (2 banks each) for the second matmul
N_CORES = 8

_NC = None


def _build_nc():
    import concourse.bacc as bacc
    import concourse.mybir as mybir
    import concourse.tile as tile
    from concourse.masks import make_identity

    f32 = mybir.dt.float32
    bf16 = mybir.dt.bfloat16
    fp8 = mybir.dt.float8e4
    AF = mybir.ActivationFunctionType
    ALU = mybir.AluOpType
    AX = mybir.AxisListType
    DR = mybir.MatmulPerfMode.DoubleRow

    nc = bacc.Bacc(
        "TRN2",
        target_bir_lowering=False,
        debug=False,
        num_devices=N_CORES,
        num_swdge_queues=4,
    )
    x = nc.dram_tensor("x", (S, C, HW), f32, kind="ExternalInput").ap()
    gamma = nc.dram_tensor("gamma", (1,), f32, kind="ExternalInput").ap()
    out = nc.dram_tensor("out", (S, C, HW), f32, kind="ExternalOutput").ap()

    with tile.TileContext(nc) as tc, ExitStack() as ctx:
        const = ctx.enter_context(tc.tile_pool(name="const", bufs=1))
        ident = const.tile([P, P], f32)
        make_identity(nc, ident)
        identb = const.tile([P, P], bf16)
        make_identity(nc, identb)
        ident8 = const.tile([P, P], fp8)
        make_identity(nc, ident8)
        gamma_sb = const.tile([P, 1], f32)
        nc.sync.dma_start(out=gamma_sb, in_=gamma.to_broadcast((P, 1)))

        xf_pool = ctx.enter_context(tc.tile_pool(name="xf_pool", bufs=16))
        vb_pool = ctx.enter_context(tc.tile_pool(name="vb_pool", bufs=4))
        vt_pool = ctx.enter_context(tc.tile_pool(name="vt_pool", bufs=NT + 2))
        att_pool = ctx.enter_context(tc.tile_pool(name="att_pool", bufs=CB + 1))
        attT_pool = ctx.enter_context(tc.tile_pool(name="attT_pool", bufs=2))
        exp_pool = ctx.enter_context(tc.tile_pool(name="exp_pool", bufs=2))
        small = ctx.enter_context(tc.tile_pool(name="small", bufs=12))
        r1_pool = ctx.enter_context(tc.tile_pool(name="r1_pool", bufs=10))
        # one shared PSUM pool: [P,1024] f32 bufs (2 banks each) = all 8 banks
        psum = ctx.enter_context(tc.tile_pool(name="psum", bufs=4, space="PSUM"))

        # per-sample state
        xh = [[[None, None] for _ in range(CB)] for _ in range(S)]
        vb2 = [[None] * (CB // 2) for _ in range(S)]
        vT2 = [[None] * NT for _ in range(S)]
        att8 = [[None] * CB for _ in range(S)]
        r1s = [[None] * CB for _ in range(S)]
        attT2 = [[None] * (CB // 2) for _ in range(S)]

        def loads(s, quarters):
            for h in range(2):
                for i in range(CB):
                    xt = xf_pool.tile([P, HHW], f32, tag="xf", name=f"xf_{s}_{i}_{h}")
                    if quarters:
                        # quarter-granularity so the first transposes start
                        # as soon as the first 0.5 MB per row-block lands
                        for q in range(2):
                            nc.sync.dma_start(
                                out=xt[:, q * (HHW // 2) : (q + 1) * (HHW // 2)],
                                in_=x[
                                    s,
                                    i * P : (i + 1) * P,
                                    h * HHW + q * (HHW // 2) : h * HHW + (q + 1) * (HHW // 2),
                                ],
                            )
                    else:
                        nc.sync.dma_start(
                            out=xt,
                            in_=x[s, i * P : (i + 1) * P, h * HHW : (h + 1) * HHW],
                        )
                    xh[s][i][h] = xt

        def vb_casts(s):
            # v in fp8, paired over channel chunks: vb2[s][u][:, ko, :] = v rows
            # of chunk 2u+ko.  f32 -> fp8 casts on the GpSimd engine (keeps this
            # 21 MB of SBUF->SBUF traffic off the DMA queues).
            for u in range(CB // 2):
                vt_ = vb_pool.tile([P, 2, HW], fp8, tag="vb", name=f"vb2_{s}_{u}")
                for ko in range(2):
                    i = 2 * u + ko
                    for h in range(2):
                        dst = vt_[:, ko, h * HHW : (h + 1) * HHW]
                        nc.gpsimd.tensor_copy(dst, xh[s][i][h])
                vb2[s][u] = vt_

        evict_ctr = [0]

        def evict(dst, src):
            # PSUM -> SBUF eviction copy, alternating DVE / ACT (2:1 — ACT is
            # pinned by the exps, DVE has more headroom)
            k = evict_ctr[0]
            evict_ctr[0] += 1
            if k % 3 == 2:
                nc.scalar.copy(dst, src)
            else:
                nc.vector.tensor_copy(dst, src)

        def v_transpose_pair_f32(s, t):
            # vT pair t (n-part, c-free) fp8 from fp32 x tiles: PE transpose,
            # fp8 cast during the single [P,1024] PSUM->SBUF eviction.
            vt_ = vt_pool.tile([P, 2, C], fp8, tag="vt", name=f"vT2_{s}_{t}")
            pt = psum.tile([P, 2, C], f32, tag="ps", name=f"ptv_{s}_{t}")
            for ko in range(2):
                k = 2 * t + ko
                h, kk = divmod(k, NB // 2)
                for i in range(CB):
                    nc.tensor.transpose(
                        pt[:, ko, i * P : (i + 1) * P],
                        xh[s][i][h][:, kk * P : (kk + 1) * P],
                        ident,
                    )
            evict(vt_, pt)
            vT2[s][t] = vt_

        def v_transpose_pair_fp8(s, t):
            # vT pair t from the fp8 vb tiles: FWL weight loads + fp8 moving,
            # less than half the PE time of the fp32 variant.
            # regular fp8 matmul against the identity: out = v_slice.T @ I is
            # an exact transpose into f32 PSUM, and unlike is_transpose=True
            # it has no fp8 stride-2 output constraint (FWL still applies).
            vt_ = vt_pool.tile([P, 2, C], fp8, tag="vt", name=f"vT2_{s}_{t}")
            pt = psum.tile([P, 2, C], f32, tag="ps", name=f"ptv8_{s}_{t}")
            for ko in range(2):
                k = 2 * t + ko
                for i in range(CB):
                    nc.tensor.matmul(
                        pt[:, ko, i * P : (i + 1) * P],
                        lhsT=vb2[s][i // 2][:, i % 2, k * P : (k + 1) * P],
                        rhs=ident8,
                        start=True,
                        stop=True,
                    )
            evict(vt_, pt)
            vT2[s][t] = vt_

        def softmax1_tail(s, i, E):
            m = small.tile([P, 1], f32, tag="sm", name=f"m_{s}_{i}")
            nc.vector.tensor_reduce(m, E, axis=AX.X, op=ALU.min)
            a = att_pool.tile([P, C], bf16, tag="att", name=f"att_{s}_{i}")
            z1 = small.tile([P, 1], f32, tag="sm", name=f"z1_{s}_{i}")
            nc.scalar.activation(a, E, AF.Exp, bias=m, scale=-1.0, accum_out=z1)
            r1 = r1_pool.tile([P, 1], f32, tag="r1", name=f"r1_{s}_{i}")
            nc.vector.reciprocal(r1, z1)
            att8[s][i] = a
            r1s[s][i] = r1

        def mm1_block(s, i, Eout, t):
            nc.tensor.matmul(
                Eout,
                lhsT=vT2[s][t][:, :, i * P : (i + 1) * P],
                rhs=vT2[s][t],
                perf_mode=DR,
                start=(t == 0),
                stop=(t == NT - 1),
            )

        def front_fused(s, fp8_trans):
            # transposes interleaved with mm1 accumulation of row-blocks 0,1
            # (software-pipelined: mm1 for pair t-1 runs while pair t's
            # eviction is in flight); row-blocks 2,3 in a second pass.
            E01 = psum.tile([P, 2, C], f32, tag="ps", name=f"E01_{s}")
            for t in range(NT):
                if fp8_trans:
                    v_transpose_pair_fp8(s, t)
                else:
                    v_transpose_pair_f32(s, t)
                if t >= 1:
                    for i in range(2):
                        mm1_block(s, i, E01[:, i, :], t - 1)
            for i in range(2):
                mm1_block(s, i, E01[:, i, :], NT - 1)
            for i in range(2):
                softmax1_tail(s, i, E01[:, i, :])
            E23 = psum.tile([P, 2, C], f32, tag="ps", name=f"E23_{s}")
            for t in range(NT):
                for i in range(2):
                    mm1_block(s, 2 + i, E23[:, i, :], t)
            for i in range(2):
                softmax1_tail(s, 2 + i, E23[:, i, :])

        def mm1_softmax1(s):
            # standalone mm1 (vT tiles already built): two row-block pairs
            for i0 in range(0, CB, 2):
                Ep = psum.tile([P, 2, C], f32, tag="ps", name=f"E_{s}_{i0}")
                for t in range(NT):
                    for j in range(2):
                        mm1_block(s, i0 + j, Ep[:, j, :], t)
                for j in range(2):
                    softmax1_tail(s, i0 + j, Ep[:, j, :])

        def att_transposes(s):
            # attT pairs (col-part, row-free) fp8 via bf16 PE transpose,
            # one [P,1024] eviction per pair
            for u in range(CB // 2):
                st = attT_pool.tile([P, 2, C], fp8, tag="attT", name=f"attT2_{s}_{u}")
                pt = psum.tile([P, 2, C], bf16, tag="ps", name=f"pta_{s}_{u}")
                for ko in range(2):
                    j = 2 * u + ko
                    for i in range(CB):
                        nc.tensor.transpose(
                            pt[:, ko, i * P : (i + 1) * P],
                            att8[s][i][:, j * P : (j + 1) * P],
                            identb,
                        )
                evict(st, pt)
                attT2[s][u] = st

        def mm2_final(s, i):
            # o = att @ v (DoubleRow), softmax over HW (with 1/Z1 folded into
            # the exp scale), then out = x + (gamma/Z2)*exp and store.
            # Final adds split DVE (h=0) / GpSimd (h=1) so stores flow early.
            er = exp_pool.tile([P, HW], bf16, tag="er", name=f"er_{s}_{i}")
            z2p = small.tile([P, NJ], f32, tag="z2p", name=f"z2p_{s}_{i}")
            for nj in range(NJ):
                o2 = psum.tile([P, 1024], f32, tag="ps", name=f"o2_{s}_{i}_{nj}")
                for hh in range(2):
                    sl = slice(nj * 1024 + hh * 512, nj * 1024 + (hh + 1) * 512)
                    for u in range(CB // 2):
                        nc.tensor.matmul(
                            o2[:, hh * 512 : (hh + 1) * 512],
                            lhsT=attT2[s][u][:, :, i * P : (i + 1) * P],
                            rhs=vb2[s][u][:, :, sl],
                            perf_mode=DR,
                            start=(u == 0),
                            stop=(u == CB // 2 - 1),
                        )
                nc.scalar.activation(
                    er[:, nj * 1024 : (nj + 1) * 1024],
                    o2,
                    AF.Exp,
                    scale=r1s[s][i],
                    accum_out=z2p[:, nj : nj + 1],
                )
            z2 = small.tile([P, 1], f32, tag="sm", name=f"z2_{s}_{i}")
            nc.vector.reduce_sum(z2, z2p, axis=AX.X)
            r2 = small.tile([P, 1], f32, tag="sm", name=f"r2_{s}_{i}")
            nc.vector.reciprocal(r2, z2)
            gz = small.tile([P, 1], f32, tag="sm", name=f"gz_{s}_{i}")
            nc.vector.tensor_scalar_mul(gz, r2, gamma_sb)
            for h in range(2):
                xt = xh[s][i][h]
                if h == 0:
                    nc.vector.scalar_tensor_tensor(
                        out=xt,
                        in0=er[:, :HHW],
                        scalar=gz,
                        in1=xt,
                        op0=ALU.mult,
                        op1=ALU.add,
                    )
                else:
                    # GpSimd can't do scalar_tensor_tensor (no TensorScalarPtr
                    # on Pool): scale er in place on DVE (4x tensor_scalar),
                    # then plain add on GpSimd.
                    nc.vector.tensor_scalar_mul(er[:, HHW:], er[:, HHW:], gz)
                    nc.gpsimd.tensor_tensor(
                        out=xt, in0=er[:, HHW:], in1=xt, op=ALU.add
                    )
                nc.sync.dma_start(
                    out=out[s, i * P : (i + 1) * P, h * HHW : (h + 1) * HHW],
                    in_=xt,
                )

        # ---- software pipeline across the two samples ----
        loads(0, quarters=True)
        loads(1, quarters=False)
        vb_casts(0)
        front_fused(0, fp8_trans=False)
        att_transposes(0)
        vb_casts(1)
        for i in range(CB):
            mm2_final(0, i)
            for t in range(i * (NT // CB), (i + 1) * (NT // CB)):
                v_transpose_pair_fp8(1, t)
        mm1_softmax1(1)
        att_transposes(1)
        for i in range(CB):
            mm2_final(1, i)

    nc.compile()
    return nc


def get_nc():
    global _NC
    if _NC is None:
        _NC = _build_nc()
    return _NC


def kernel(x: np.ndarray, gamma: np.ndarray) -> np.ndarray:
    from concourse.bass_utils import run_bass_kernel_spmd

    B, Cx, H, W = x.shape
    assert (B, Cx, H * W) == (16, C, HW), (B, Cx, H, W)
    nc = get_nc()
    xs = np.ascontiguousarray(np.asarray(x, dtype=np.float32)).reshape(B, Cx, H * W)
    g = np.ascontiguousarray(np.asarray(gamma, dtype=np.float32)).reshape(1)
    in_maps = [{"x": xs[S * c : S * (c + 1)], "gamma": g} for c in range(N_CORES)]
    res = run_bass_kernel_spmd(nc, in_maps, core_ids=list(range(N_CORES)))
    out = np.concatenate([res.results[c]["out"] for c in range(N_CORES)], axis=0)
    return out.reshape(B, Cx, H, W).astype(np.float32)


# revision 5
# speedup vs baseline: 1.3882x; 1.3882x over previous
"""CAM (channel attention) module kernel for Trainium2, 8-core data-parallel.

Reference computation (per sample, C=512, HW=4096):
    v = x.reshape(C, HW)
    E = v @ v.T                                  # (C, C)
    att = softmax(rowmax(E) - E, axis=-1)        # == softmax(-E) stabilized at rowmin
    o = att @ v                                  # (C, HW)
    o = softmax(o, axis=-1)
    out = x + gamma * o
Sharding: data-parallel over batch B=16 -> 2 samples per NeuronCore.

v3: bf16 DRAM I/O.  The tolerance (2e-2) comfortably admits bf16-rounded
inputs/outputs (~2e-3), so the host casts x to bf16 and upcasts the output;
HBM traffic drops from 33.6 MB to ~25 MB per core:
- x lands in SBUF as bf16 (half the load bytes, half the SBUF);
- vb (natural-layout fp8 v for matmul 2 / the transposes) is produced by
  SWDGE cast-load DMAs straight from DRAM (bf16 -> fp8), using spare DMA
  bandwidth instead of engine time;
- all v^T transposes are fp8 identity-matmuls (FWL weight loads) feeding
  DoubleRow matmul 1; transpose pairs land in one [P,1024] PSUM tile and
  evict with a single copy;
- the final out = x + (gamma/Z2)*exp is an all-bf16 scalar_tensor_tensor on
  DVE (2x packed mode), stores are bf16;
- one shared PSUM pool ([P,1024] f32, bufs=4 = all 8 banks).
"""

import sys

if "/opt/trn_rl_repo" not in sys.path:
    sys.path.insert(0, "/opt/trn_rl_repo")

from contextlib import ExitStack

import numpy as np

P = 128
C = 512
HW = 4096
HHW = HW // 2  # 2048: half-width x tiles
S = 2  # samples per core
CB = C // P  # 4 channel blocks
NB = HW // P  # 32 spatial blocks
NT = NB // 2  # 16 DoubleRow k-pairs for matmul 1
NJ = HW // 1024  # 4 psum chunks (2 banks each) for the second matmul
N_CORES = 8

_NC = None


def _build_nc():
    import concourse.bacc as bacc
    import concourse.mybir as mybir
    import concourse.tile as tile
    from concourse.masks import make_identity

    f32 = mybir.dt.float32
    bf16 = mybir.dt.bfloat16
    fp8 = mybir.dt.float8e4
    AF = mybir.ActivationFunctionType
    ALU = mybir.AluOpType
    AX = mybir.AxisListType
    DR = mybir.MatmulPerfMode.DoubleRow

    nc = bacc.Bacc(
        "TRN2",
        target_bir_lowering=False,
        debug=False,
        num_devices=N_CORES,
        num_swdge_queues=4,
    )
    x = nc.dram_tensor("x", (S, C, HW), bf16, kind="ExternalInput").ap()
    gamma = nc.dram_tensor("gamma", (1,), f32, kind="ExternalInput").ap()
    out = nc.dram_tensor("out", (S, C, HW), bf16, kind="ExternalOutput").ap()

    with tile.TileContext(nc) as tc, ExitStack() as ctx:
        const = ctx.enter_context(tc.tile_pool(name="const", bufs=1))
        identb = const.tile([P, P], bf16)
        make_identity(nc, identb)
        ident8 = const.tile([P, P], fp8)
        make_identity(nc, ident8)
        gamma_sb = const.tile([P, 1], f32)
        nc.sync.dma_start(out=gamma_sb, in_=gamma.to_broadcast((P, 1)))

        xf_pool = ctx.enter_context(tc.tile_pool(name="xf_pool", bufs=16))
        vb_pool = ctx.enter_context(tc.tile_pool(name="vb_pool", bufs=4))
        vt_pool = ctx.enter_context(tc.tile_pool(name="vt_pool", bufs=NT + 2))
        att_pool = ctx.enter_context(tc.tile_pool(name="att_pool", bufs=CB + 1))
        attT_pool = ctx.enter_context(tc.tile_pool(name="attT_pool", bufs=2))
        exp_pool = ctx.enter_context(tc.tile_pool(name="exp_pool", bufs=3))
        small = ctx.enter_context(tc.tile_pool(name="small", bufs=12))
        r1_pool = ctx.enter_context(tc.tile_pool(name="r1_pool", bufs=10))
        # one shared PSUM pool: [P,1024] f32 bufs (2 banks each) = all 8 banks
        psum = ctx.enter_context(tc.tile_pool(name="psum", bufs=4, space="PSUM"))

        # per-sample state
        xh = [[[None, None] for _ in range(CB)] for _ in range(S)]
        vb2 = [[None] * (CB // 2) for _ in range(S)]
        vT2 = [[None] * NT for _ in range(S)]
        att8 = [[None] * CB for _ in range(S)]
        r1s = [[None] * CB for _ in range(S)]
        attT2 = [[None] * (CB // 2) for _ in range(S)]

        def vb_loads(s, quarters):
            # vb2[s][u][:, ko, :] = fp8 rows of channel chunk 2u+ko, cast
            # bf16 -> fp8 by SWDGE DMAs straight from DRAM.
            nq = 4 if quarters else 1
            for u in range(CB // 2):
                vt_ = vb_pool.tile([P, 2, HW], fp8, tag="vb", name=f"vb2_{s}_{u}")
                vb2[s][u] = vt_
            # emit in k-major quarter order so the first transposes start early
            for q in range(nq):
                qs = slice(q * (HW // nq), (q + 1) * (HW // nq))
                for u in range(CB // 2):
                    for ko in range(2):
                        i = 2 * u + ko
                        nc.gpsimd.dma_start(
                            out=vb2[s][u][:, ko, qs],
                            in_=x[s, i * P : (i + 1) * P, qs],
                        )

        def x_loads(s):
            # bf16 x tiles, only consumed by the final residual add
            for h in range(2):
                for i in range(CB):
                    xt = xf_pool.tile([P, HHW], bf16, tag="xf", name=f"xf_{s}_{i}_{h}")
                    nc.sync.dma_start(
                        out=xt,
                        in_=x[s, i * P : (i + 1) * P, h * HHW : (h + 1) * HHW],
                    )
                    xh[s][i][h] = xt

        evict_ctr = [0]

        def evict(dst, src):
            # PSUM -> SBUF eviction copy, alternating DVE / ACT (2:1 — ACT is
            # pinned by the exps, DVE has more headroom)
            k = evict_ctr[0]
            evict_ctr[0] += 1
            if k % 3 == 2:
                nc.scalar.copy(dst, src)
            else:
                nc.vector.tensor_copy(dst, src)

        def v_transpose_pair(s, t):
            # vT pair t (n-part, c-free) fp8: regular fp8 matmul against the
            # identity (out = v_slice.T @ I) — exact transpose into f32 PSUM,
            # FWL weight loads, no fp8 is_transpose stride-2 constraint.
            vt_ = vt_pool.tile([P, 2, C], fp8, tag="vt", name=f"vT2_{s}_{t}")
            pt = psum.tile([P, 2, C], f32, tag="ps", name=f"ptv_{s}_{t}")
            for ko in range(2):
                k = 2 * t + ko
                for i in range(CB):
                    nc.tensor.matmul(
                        pt[:, ko, i * P : (i + 1) * P],
                        lhsT=vb2[s][i // 2][:, i % 2, k * P : (k + 1) * P],
                        rhs=ident8,
                        start=True,
                        stop=True,
                    )
            evict(vt_, pt)
            vT2[s][t] = vt_

        def softmax1_tail(s, i, E):
            m = small.tile([P, 1], f32, tag="sm", name=f"m_{s}_{i}")
            nc.vector.tensor_reduce(m, E, axis=AX.X, op=ALU.min)
            a = att_pool.tile([P, C], bf16, tag="att", name=f"att_{s}_{i}")
            z1 = small.tile([P, 1], f32, tag="sm", name=f"z1_{s}_{i}")
            nc.scalar.activation(a, E, AF.Exp, bias=m, scale=-1.0, accum_out=z1)
            r1 = r1_pool.tile([P, 1], f32, tag="r1", name=f"r1_{s}_{i}")
            nc.vector.reciprocal(r1, z1)
            att8[s][i] = a
            r1s[s][i] = r1

        def mm1_block(s, i, Eout, t):
            nc.tensor.matmul(
                Eout,
                lhsT=vT2[s][t][:, :, i * P : (i + 1) * P],
                rhs=vT2[s][t],
                perf_mode=DR,
                start=(t == 0),
                stop=(t == NT - 1),
            )

        def front_fused(s):
            # transposes interleaved with mm1 accumulation of row-blocks 0,1
            # (software-pipelined: mm1 for pair t-1 runs while pair t's
            # eviction is in flight); row-blocks 2,3 in a second pass.
            E01 = psum.tile([P, 2, C], f32, tag="ps", name=f"E01_{s}")
            for t in range(NT):
                v_transpose_pair(s, t)
                if t >= 1:
                    for i in range(2):
                        mm1_block(s, i, E01[:, i, :], t - 1)
            for i in range(2):
                mm1_block(s, i, E01[:, i, :], NT - 1)
            for i in range(2):
                softmax1_tail(s, i, E01[:, i, :])
            E23 = psum.tile([P, 2, C], f32, tag="ps", name=f"E23_{s}")
            for t in range(NT):
                for i in range(2):
                    mm1_block(s, 2 + i, E23[:, i, :], t)
            for i in range(2):
                softmax1_tail(s, 2 + i, E23[:, i, :])

        def mm1_softmax1(s):
            # standalone mm1 (vT tiles already built): two row-block pairs
            for i0 in range(0, CB, 2):
                Ep = psum.tile([P, 2, C], f32, tag="ps", name=f"E_{s}_{i0}")
                for t in range(NT):
                    for j in range(2):
                        mm1_block(s, i0 + j, Ep[:, j, :], t)
                for j in range(2):
                    softmax1_tail(s, i0 + j, Ep[:, j, :])

        def att_transposes(s):
            # attT pairs (col-part, row-free) fp8 via bf16 PE transpose,
            # one [P,1024] eviction per pair
            for u in range(CB // 2):
                st = attT_pool.tile([P, 2, C], fp8, tag="attT", name=f"attT2_{s}_{u}")
                pt = psum.tile([P, 2, C], bf16, tag="ps", name=f"pta_{s}_{u}")
                for ko in range(2):
                    j = 2 * u + ko
                    for i in range(CB):
                        nc.tensor.transpose(
                            pt[:, ko, i * P : (i + 1) * P],
                            att8[s][i][:, j * P : (j + 1) * P],
                            identb,
                        )
                evict(st, pt)
                attT2[s][u] = st

        def mm2_final(s, i):
            # o = att @ v (DoubleRow), softmax over HW (with 1/Z1 folded into
            # the exp scale), then out = x + (gamma/Z2)*exp (all-bf16 STT on
            # DVE, 2x packed mode) and bf16 store.
            er = exp_pool.tile([P, HW], bf16, tag="er", name=f"er_{s}_{i}")
            z2p = small.tile([P, NJ], f32, tag="z2p", name=f"z2p_{s}_{i}")
            for nj in range(NJ):
                o2 = psum.tile([P, 1024], f32, tag="ps", name=f"o2_{s}_{i}_{nj}")
                for hh in range(2):
                    sl = slice(nj * 1024 + hh * 512, nj * 1024 + (hh + 1) * 512)
                    for u in range(CB // 2):
                        nc.tensor.matmul(
                            o2[:, hh * 512 : (hh + 1) * 512],
                            lhsT=attT2[s][u][:, :, i * P : (i + 1) * P],
                            rhs=vb2[s][u][:, :, sl],
                            perf_mode=DR,
                            start=(u == 0),
                            stop=(u == CB // 2 - 1),
                        )
                nc.scalar.activation(
                    er[:, nj * 1024 : (nj + 1) * 1024],
                    o2,
                    AF.Exp,
                    scale=r1s[s][i],
                    accum_out=z2p[:, nj : nj + 1],
                )
            z2 = small.tile([P, 1], f32, tag="sm", name=f"z2_{s}_{i}")
            nc.vector.reduce_sum(z2, z2p, axis=AX.X)
            r2 = small.tile([P, 1], f32, tag="sm", name=f"r2_{s}_{i}")
            nc.vector.reciprocal(r2, z2)
            gz = small.tile([P, 1], f32, tag="sm", name=f"gz_{s}_{i}")
            nc.vector.tensor_scalar_mul(gz, r2, gamma_sb)
            for h in range(2):
                xt = xh[s][i][h]
                nc.vector.scalar_tensor_tensor(
                    out=xt,
                    in0=er[:, h * HHW : (h + 1) * HHW],
                    scalar=gz,
                    in1=xt,
                    op0=ALU.mult,
                    op1=ALU.add,
                )
                nc.sync.dma_start(
                    out=out[s, i * P : (i + 1) * P, h * HHW : (h + 1) * HHW],
                    in_=xt,
                )

        # ---- software pipeline across the two samples ----
        vb_loads(0, quarters=True)
        x_loads(0)
        vb_loads(1, quarters=False)
        x_loads(1)
        front_fused(0)
        att_transposes(0)
        for i in range(CB):
            mm2_final(0, i)
            for t in range(i * (NT // CB), (i + 1) * (NT // CB)):
                v_transpose_pair(1, t)
        mm1_softmax1(1)
        att_transposes(1)
        for i in range(CB):
            mm2_final(1, i)

    nc.compile()
    return nc


def get_nc():
    global _NC
    if _NC is None:
        _NC = _build_nc()
    return _NC


def kernel(x: np.ndarray, gamma: np.ndarray) -> np.ndarray:
    import ml_dtypes
    from concourse.bass_utils import run_bass_kernel_spmd

    B, Cx, H, W = x.shape
    assert (B, Cx, H * W) == (16, C, HW), (B, Cx, H, W)
    nc = get_nc()
    xs = np.ascontiguousarray(
        np.asarray(x, dtype=np.float32).reshape(B, Cx, H * W).astype(ml_dtypes.bfloat16)
    )
    g = np.ascontiguousarray(np.asarray(gamma, dtype=np.float32)).reshape(1)
    in_maps = [{"x": xs[S * c : S * (c + 1)], "gamma": g} for c in range(N_CORES)]
    res = run_bass_kernel_spmd(nc, in_maps, core_ids=list(range(N_CORES)))
    out = np.concatenate([res.results[c]["out"] for c in range(N_CORES)], axis=0)
    return out.astype(np.float32).reshape(B, Cx, H, W)


# revision 10
# speedup vs baseline: 1.4967x; 1.0781x over previous
"""CAM (channel attention) module kernel for Trainium2, 8-core data-parallel.

Reference computation (per sample, C=512, HW=4096):
    v = x.reshape(C, HW)
    E = v @ v.T                                  # (C, C)
    att = softmax(rowmax(E) - E, axis=-1)        # == softmax(-E) stabilized at rowmin
    o = att @ v                                  # (C, HW)
    o = softmax(o, axis=-1)
    out = x + gamma * o
Sharding: data-parallel over batch B=16 -> 2 samples per NeuronCore.

v3: bf16 DRAM I/O.  The tolerance (2e-2) comfortably admits bf16-rounded
inputs/outputs (~2e-3), so the host casts x to bf16 and upcasts the output;
HBM traffic drops from 33.6 MB to ~25 MB per core:
- x lands in SBUF as bf16 (half the load bytes, half the SBUF);
- vb (natural-layout fp8 v for matmul 2 / the transposes) is produced by
  SWDGE cast-load DMAs straight from DRAM (bf16 -> fp8), using spare DMA
  bandwidth instead of engine time;
- all v^T transposes are fp8 identity-matmuls (FWL weight loads) feeding
  DoubleRow matmul 1; transpose pairs land in one [P,1024] PSUM tile and
  evict with a single copy;
- the final out = x + (gamma/Z2)*exp is an all-bf16 scalar_tensor_tensor on
  DVE (2x packed mode), stores are bf16;
- one shared PSUM pool ([P,1024] f32, bufs=4 = all 8 banks).
"""

import sys

if "/opt/trn_rl_repo" not in sys.path:
    sys.path.insert(0, "/opt/trn_rl_repo")

from contextlib import ExitStack

import numpy as np

P = 128
C = 512
HW = 4096
HHW = HW // 2  # 2048: half-width x tiles
S = 2  # samples per core
CB = C // P  # 4 channel blocks
NB = HW // P  # 32 spatial blocks
NT = NB // 2  # 16 DoubleRow k-pairs for matmul 1
NJ = HW // 1024  # 4 psum chunks (2 banks each) for the second matmul
N_CORES = 8

_NC = None


def _build_nc():
    import concourse.bacc as bacc
    import concourse.mybir as mybir
    import concourse.tile as tile
    from concourse.masks import make_identity

    f32 = mybir.dt.float32
    bf16 = mybir.dt.bfloat16
    fp8 = mybir.dt.float8e4
    AF = mybir.ActivationFunctionType
    ALU = mybir.AluOpType
    AX = mybir.AxisListType
    DR = mybir.MatmulPerfMode.DoubleRow

    nc = bacc.Bacc(
        "TRN2",
        target_bir_lowering=False,
        debug=False,
        num_devices=N_CORES,
        num_swdge_queues=4,
    )
    x = nc.dram_tensor("x", (S, C, HW), bf16, kind="ExternalInput").ap()
    gamma = nc.dram_tensor("gamma", (1,), f32, kind="ExternalInput").ap()
    out = nc.dram_tensor("out", (S, C, HW), bf16, kind="ExternalOutput").ap()

    with tile.TileContext(nc) as tc, ExitStack() as ctx:
        const = ctx.enter_context(tc.tile_pool(name="const", bufs=1))
        identb = const.tile([P, P], bf16)
        make_identity(nc, identb)
        ident8 = const.tile([P, P], fp8)
        make_identity(nc, ident8)
        gamma_sb = const.tile([P, 1], f32)
        nc.sync.dma_start(out=gamma_sb, in_=gamma.to_broadcast((P, 1)))

        xf_pool = ctx.enter_context(tc.tile_pool(name="xf_pool", bufs=16))
        vb_pool = ctx.enter_context(tc.tile_pool(name="vb_pool", bufs=4))
        vt_pool = ctx.enter_context(tc.tile_pool(name="vt_pool", bufs=NT + 2))
        att_pool = ctx.enter_context(tc.tile_pool(name="att_pool", bufs=CB + 1))
        attT_pool = ctx.enter_context(tc.tile_pool(name="attT_pool", bufs=2))
        exp_pool = ctx.enter_context(tc.tile_pool(name="exp_pool", bufs=3))
        small = ctx.enter_context(tc.tile_pool(name="small", bufs=12))
        r1_pool = ctx.enter_context(tc.tile_pool(name="r1_pool", bufs=10))
        # one shared PSUM pool: [P,1024] f32 bufs (2 banks each) = all 8 banks
        psum = ctx.enter_context(tc.tile_pool(name="psum", bufs=4, space="PSUM"))

        # per-sample state
        xh = [[[None, None] for _ in range(CB)] for _ in range(S)]
        vb2 = [[None] * (CB // 2) for _ in range(S)]
        vT2 = [[None] * NT for _ in range(S)]
        att8 = [[None] * CB for _ in range(S)]
        r1s = [[None] * CB for _ in range(S)]
        attT2 = [[None] * (CB // 2) for _ in range(S)]

        def vb_loads(s, quarters):
            # vb2[s][u][:, ko, :] = fp8 rows of channel chunk 2u+ko, cast
            # bf16 -> fp8 by SWDGE DMAs straight from DRAM.
            nq = 4 if quarters else 1
            for u in range(CB // 2):
                vt_ = vb_pool.tile([P, 2, HW], fp8, tag="vb", name=f"vb2_{s}_{u}")
                vb2[s][u] = vt_
            # emit in k-major quarter order so the first transposes start early
            for q in range(nq):
                qs = slice(q * (HW // nq), (q + 1) * (HW // nq))
                for u in range(CB // 2):
                    for ko in range(2):
                        i = 2 * u + ko
                        nc.gpsimd.dma_start(
                            out=vb2[s][u][:, ko, qs],
                            in_=x[s, i * P : (i + 1) * P, qs],
                        )

        def x_loads(s):
            # bf16 x tiles, only consumed by the final residual add.  These
            # ride the same in-order SWDGE ring as the vb cast-loads so the
            # load stream drains in priority order (vb0, vb1, x0, x1) instead
            # of competing with them for DMA-engine bandwidth.
            for h in range(2):
                for i in range(CB):
                    xt = xf_pool.tile([P, HHW], bf16, tag="xf", name=f"xf_{s}_{i}_{h}")
                    nc.gpsimd.dma_start(
                        out=xt,
                        in_=x[s, i * P : (i + 1) * P, h * HHW : (h + 1) * HHW],
                    )
                    xh[s][i][h] = xt

        evict_ctr = [0]

        def evict(dst, src):
            # PSUM -> SBUF eviction copy, alternating DVE / ACT (2:1 — ACT is
            # pinned by the exps, DVE has more headroom)
            k = evict_ctr[0]
            evict_ctr[0] += 1
            if k % 3 == 2:
                nc.scalar.copy(dst, src)
            else:
                nc.vector.tensor_copy(dst, src)

        def v_transpose_pair(s, t):
            # vT pair t (n-part, c-free) fp8: regular fp8 matmul against the
            # identity (out = v_slice.T @ I) — exact transpose into f32 PSUM,
            # FWL weight loads, no fp8 is_transpose stride-2 constraint.
            vt_ = vt_pool.tile([P, 2, C], fp8, tag="vt", name=f"vT2_{s}_{t}")
            pt = psum.tile([P, 2, C], f32, tag="ps", name=f"ptv_{s}_{t}")
            for ko in range(2):
                k = 2 * t + ko
                for i in range(CB):
                    nc.tensor.matmul(
                        pt[:, ko, i * P : (i + 1) * P],
                        lhsT=vb2[s][i // 2][:, i % 2, k * P : (k + 1) * P],
                        rhs=ident8,
                        start=True,
                        stop=True,
                    )
            evict(vt_, pt)
            vT2[s][t] = vt_

        def softmax1_tail(s, i, E):
            m = small.tile([P, 1], f32, tag="sm", name=f"m_{s}_{i}")
            nc.vector.tensor_reduce(m, E, axis=AX.X, op=ALU.min)
            a = att_pool.tile([P, C], bf16, tag="att", name=f"att_{s}_{i}")
            z1 = small.tile([P, 1], f32, tag="sm", name=f"z1_{s}_{i}")
            nc.scalar.activation(a, E, AF.Exp, bias=m, scale=-1.0, accum_out=z1)
            r1 = r1_pool.tile([P, 1], f32, tag="r1", name=f"r1_{s}_{i}")
            nc.vector.reciprocal(r1, z1)
            att8[s][i] = a
            r1s[s][i] = r1

        def mm1_block(s, i, Eout, t):
            nc.tensor.matmul(
                Eout,
                lhsT=vT2[s][t][:, :, i * P : (i + 1) * P],
                rhs=vT2[s][t],
                perf_mode=DR,
                start=(t == 0),
                stop=(t == NT - 1),
            )

        def front_fused(s):
            # transposes interleaved with mm1 accumulation of row-blocks 0,1
            # (software-pipelined: mm1 for pair t-1 runs while pair t's
            # eviction is in flight); row-blocks 2,3 in a second pass.
            E01 = psum.tile([P, 2, C], f32, tag="ps", name=f"E01_{s}")
            for t in range(NT):
                v_transpose_pair(s, t)
                if t >= 1:
                    for i in range(2):
                        mm1_block(s, i, E01[:, i, :], t - 1)
            for i in range(2):
                mm1_block(s, i, E01[:, i, :], NT - 1)
            for i in range(2):
                softmax1_tail(s, i, E01[:, i, :])
            E23 = psum.tile([P, 2, C], f32, tag="ps", name=f"E23_{s}")
            for t in range(NT):
                for i in range(2):
                    mm1_block(s, 2 + i, E23[:, i, :], t)
            for i in range(2):
                softmax1_tail(s, 2 + i, E23[:, i, :])

        def mm1_softmax1(s):
            # standalone mm1 (vT tiles already built): two row-block pairs
            for i0 in range(0, CB, 2):
                Ep = psum.tile([P, 2, C], f32, tag="ps", name=f"E_{s}_{i0}")
                for t in range(NT):
                    for j in range(2):
                        mm1_block(s, i0 + j, Ep[:, j, :], t)
                for j in range(2):
                    softmax1_tail(s, i0 + j, Ep[:, j, :])

        def att_transposes(s):
            # attT pairs (col-part, row-free) fp8 via bf16 PE transpose,
            # one [P,1024] eviction per pair
            for u in range(CB // 2):
                st = attT_pool.tile([P, 2, C], fp8, tag="attT", name=f"attT2_{s}_{u}")
                pt = psum.tile([P, 2, C], bf16, tag="ps", name=f"pta_{s}_{u}")
                for ko in range(2):
                    j = 2 * u + ko
                    for i in range(CB):
                        nc.tensor.transpose(
                            pt[:, ko, i * P : (i + 1) * P],
                            att8[s][i][:, j * P : (j + 1) * P],
                            identb,
                        )
                evict(st, pt)
                attT2[s][u] = st

        def mm2_final(s, i):
            # o = att @ v (DoubleRow), softmax over HW (with 1/Z1 folded into
            # the exp scale), then out = x + (gamma/Z2)*exp (all-bf16 STT on
            # DVE, 2x packed mode) and bf16 store.
            er = exp_pool.tile([P, HW], bf16, tag="er", name=f"er_{s}_{i}")
            z2p = small.tile([P, NJ], f32, tag="z2p", name=f"z2p_{s}_{i}")
            for nj in range(NJ):
                o2 = psum.tile([P, 1024], f32, tag="ps", name=f"o2_{s}_{i}_{nj}")
                for hh in range(2):
                    sl = slice(nj * 1024 + hh * 512, nj * 1024 + (hh + 1) * 512)
                    for u in range(CB // 2):
                        nc.tensor.matmul(
                            o2[:, hh * 512 : (hh + 1) * 512],
                            lhsT=attT2[s][u][:, :, i * P : (i + 1) * P],
                            rhs=vb2[s][u][:, :, sl],
                            perf_mode=DR,
                            start=(u == 0),
                            stop=(u == CB // 2 - 1),
                        )
                nc.scalar.activation(
                    er[:, nj * 1024 : (nj + 1) * 1024],
                    o2,
                    AF.Exp,
                    scale=r1s[s][i],
                    accum_out=z2p[:, nj : nj + 1],
                )
            z2 = small.tile([P, 1], f32, tag="sm", name=f"z2_{s}_{i}")
            nc.vector.reduce_sum(z2, z2p, axis=AX.X)
            r2 = small.tile([P, 1], f32, tag="sm", name=f"r2_{s}_{i}")
            nc.vector.reciprocal(r2, z2)
            gz = small.tile([P, 1], f32, tag="sm", name=f"gz_{s}_{i}")
            nc.vector.tensor_scalar_mul(gz, r2, gamma_sb)
            for h in range(2):
                xt = xh[s][i][h]
                if h == 0:
                    nc.vector.scalar_tensor_tensor(
                        out=xt,
                        in0=er[:, :HHW],
                        scalar=gz,
                        in1=xt,
                        op0=ALU.mult,
                        op1=ALU.add,
                    )
                else:
                    # DVE STT has no packed mode (measured 1x): offload half
                    # the residual adds to GpSimd (scale er in place on DVE
                    # first — Pool has no TensorScalarPtr)
                    nc.vector.tensor_scalar_mul(er[:, HHW:], er[:, HHW:], gz)
                    nc.gpsimd.tensor_tensor(
                        out=xt, in0=er[:, HHW:], in1=xt, op=ALU.add
                    )
                nc.sync.dma_start(
                    out=out[s, i * P : (i + 1) * P, h * HHW : (h + 1) * HHW],
                    in_=xt,
                )

        # ---- software pipeline across the two samples ----
        # one in-order load stream, highest priority first: vb0 gates the s0
        # front, vb1 gates the s1 transposes (interleaved into the s0 back
        # half), x0/x1 are only needed by the trailing residual adds
        vb_loads(0, quarters=True)
        vb_loads(1, quarters=True)
        x_loads(0)
        x_loads(1)
        front_fused(0)
        att_transposes(0)
        for i in range(CB):
            mm2_final(0, i)
            for t in range(i * (NT // CB), (i + 1) * (NT // CB)):
                v_transpose_pair(1, t)
        mm1_softmax1(1)
        att_transposes(1)
        for i in range(CB):
            mm2_final(1, i)

    nc.compile()
    return nc


def get_nc():
    global _NC
    if _NC is None:
        _NC = _build_nc()
    return _NC


def kernel(x: np.ndarray, gamma: np.ndarray) -> np.ndarray:
    import ml_dtypes
    from concourse.bass_utils import run_bass_kernel_spmd

    B, Cx, H, W = x.shape
    assert (B, Cx, H * W) == (16, C, HW), (B, Cx, H, W)
    nc = get_nc()
    xs = np.ascontiguousarray(
        np.asarray(x, dtype=np.float32).reshape(B, Cx, H * W).astype(ml_dtypes.bfloat16)
    )
    g = np.ascontiguousarray(np.asarray(gamma, dtype=np.float32)).reshape(1)
    in_maps = [{"x": xs[S * c : S * (c + 1)], "gamma": g} for c in range(N_CORES)]
    res = run_bass_kernel_spmd(nc, in_maps, core_ids=list(range(N_CORES)))
    out = np.concatenate([res.results[c]["out"] for c in range(N_CORES)], axis=0)
    return out.astype(np.float32).reshape(B, Cx, H, W)
